# revision 11
# baseline (speedup 1.0000x reference)
"""TT-matrix dense layer (KerasDense via tensor-train) on 8 TRN2 NeuronCores.

out[b, :] = relu(x[b, :] @ W + bias),  W = TT(core0..core3), 4096x4096.

Cores merged pairwise (s2 = middle TT bond, rank 8):
  W01[(m1 m2), s2, (n1 n2)] = sum_s1 core0[0,m1,n1,s1] * core1[s1,m2,n2,s2]
  W23[s2, (m3 m4), (n3 n4)] = sum_s3 core2[s2,m3,n3,s3] * core3[s3,m4,n4,0]
  out[b, (n12 n34)] = relu( sum_{s2,m12,m34}
        x[b, (m12 m34)] * W01[m12, s2, n12] * W23[s2, m34, n34] + bias )

Per-core dataflow (batch sharded 8 ways, 2048 rows/core, 64 chunks x 32 rows):
  x is pre-laid-out on the HOST as bf16 [128, ch, r, bl, m34] with
  partition p = half*64 + m12 (half: E rows 4r+{0,1}, O rows 4r+{2,3}),
  so the DMA in is one fully-contiguous stream per 8-chunk super-tile.

  step1 (contract m12), per round r: two row-packed K=64 matmuls
  (x-E stationary at rows 0-63, x-O at rows 64-127), both N=512
  (rhs = W01 [m12, (s2 n12)], duplicated across partition halves),
  writing the two halves of one [128, 1024] 2-bank PSUM tile (pool
  depth 3 = 6 banks so the PE runs ahead of evacuation).
  Evacuate with ONE [128,1024] fp32->bf16 copy, DVE/ACT balanced 5/11.

  step2 (contract m34, PSUM-accumulate s2): 4-quadrant tile_position
  packing -- (0,0)/(64,64) serve the E half, (0,64)/(64,0) the O half,
  all four concurrent (full PE array).  Bias is folded in as a final
  accumulating matmul: lhsT = bias[n12, n34], rhs = delta(n12) pattern.
  Relu is fused into the step2 PSUM->SBUF evacuation; output goes out
  n-major (partitions = (bl, n34)) as contiguous DMA; the HOST inverts
  the layout during the gather.
"""

import numpy as np

BATCH = 16384
NCORES = 8
B = BATCH // NCORES     # 2048 rows per core
F = 4096
R = 8                   # TT bond s2
NCH = 64                # chunks per core, 32 rows each
NRND = 8                # rounds per chunk, 4 rows each
SUP = 8                 # chunks per x super-tile (256 rows)

_compiled = None


def _build():
    from contextlib import ExitStack
    from concourse import bacc, tile, mybir

    dt = mybir.dt.float32
    db = mybir.dt.bfloat16
    nc = bacc.Bacc("TRN2", target_bir_lowering=False, debug=False)

    x_d = nc.dram_tensor("x", [128, NCH, NRND, 2, 64], db, kind="ExternalInput")
    w01_d = nc.dram_tensor("w01", [128, 512], db, kind="ExternalInput")
    w23_d = nc.dram_tensor("w23", [128, R, 64], db, kind="ExternalInput")
    biasm_d = nc.dram_tensor("biasm", [128, 2, 64], db, kind="ExternalInput")
    delta_d = nc.dram_tensor("delta", [128, R, 64], db, kind="ExternalInput")
    out_d = nc.dram_tensor("out", [NCH, 128, 1024], dt, kind="ExternalOutput")

    with tile.TileContext(nc) as tc, ExitStack() as ctx:
        const = ctx.enter_context(tc.tile_pool(name="const", bufs=1))
        xpool = ctx.enter_context(tc.tile_pool(name="x", bufs=2))
        vpool = ctx.enter_context(tc.tile_pool(name="v", bufs=3))
        opool = ctx.enter_context(tc.tile_pool(name="o", bufs=2))
        ps1 = ctx.enter_context(tc.tile_pool(name="ps1", bufs=3, space="PSUM"))
        ps2 = ctx.enter_context(tc.tile_pool(name="ps2", bufs=1, space="PSUM"))

        w01 = const.tile([128, 512], db)
        w23 = const.tile([128, R, 64], db)
        biasm = const.tile([128, 2, 64], db)
        delta = const.tile([128, R, 64], db)
        nc.sync.dma_start(w01[:], w01_d.ap())
        nc.sync.dma_start(w23[:], w23_d.ap())
        nc.sync.dma_start(biasm[:], biasm_d.ap())
        nc.sync.dma_start(delta[:], delta_d.ap())

        xv = x_d.ap()
        ov = out_d.ap()

        # prefetch super-tile 0, sliced per chunk so compute starts early
        xts = {}
        xt0 = xpool.tile([128, SUP, NRND, 2, 64], db, tag="xt")
        for i in range(SUP):
            nc.sync.dma_start(xt0[:, i], xv[:, i])
        xts[0] = xt0

        vhist = {}
        ecnt = 0  # global evac instruction counter for DVE/ACT load balance

        def evac_copy(dst, src):
            nonlocal ecnt
            # DVE copy [128,1024] ~1192ns, ACT ~997ns -> give DVE 5/11
            if (ecnt % 11) in (0, 2, 4, 6, 8):
                nc.vector.tensor_copy(dst, src)
            else:
                nc.scalar.copy(dst, src)
            ecnt += 1

        for ch in range(NCH + 2):
            if ch < NCH:
                st, chl = divmod(ch, SUP)
                if chl == 0 and st + 1 < NCH // SUP:
                    xtn = xpool.tile([128, SUP, NRND, 2, 64], db, tag="xt")
                    nc.sync.dma_start(xtn[:], xv[:, (st + 1) * SUP:(st + 2) * SUP])
                    xts[st + 1] = xtn
                xt = xts[st]
                vEO = vpool.tile([128, NRND, 1024], db, tag="vEO")
                vhist[ch] = vEO
            if ch >= 2:
                pv = vhist[ch - 2]
                v5 = pv[:].rearrange("p t (g s n) -> p g t s n", g=2, s=R)
                poEO = ps2.tile([128, 2, 512], dt, tag="poEO")

            for r in range(NRND):
                if ch < NCH:
                    # ---- step 1: two row-packed matmuls into one 2-bank tile
                    pEO = ps1.tile([128, 1024], dt, tag="pEO")
                    nc.tensor.matmul(pEO[:, 0:512], xt[0:64, chl, r],
                                     w01[0:64], start=True, stop=True,
                                     tile_position=(0, 0))
                    nc.tensor.matmul(pEO[:, 512:1024], xt[64:128, chl, r],
                                     w01[64:128], start=True, stop=True,
                                     tile_position=(64, 0))
                    evac_copy(vEO[:, r], pEO[:])
                if ch >= 2:
                    # ---- step 2: 4-quadrant packed, accumulate over s2
                    s2 = r
                    st1 = (s2 == 0)
                    nc.tensor.matmul(poEO[0:64, 0], w23[0:64, s2],
                                     v5[0:64, 0, :, s2], start=st1, stop=False,
                                     tile_position=(0, 0))
                    nc.tensor.matmul(poEO[64:128, 0], w23[64:128, s2],
                                     v5[64:128, 0, :, s2], start=st1, stop=False,
                                     tile_position=(64, 64))
                    nc.tensor.matmul(poEO[64:128, 1], w23[0:64, s2],
                                     v5[0:64, 1, :, s2], start=st1, stop=False,
                                     tile_position=(0, 64))
                    nc.tensor.matmul(poEO[0:64, 1], w23[64:128, s2],
                                     v5[64:128, 1, :, s2], start=st1, stop=False,
                                     tile_position=(64, 0))

            if ch >= 2:
                # bias via delta-pattern matmuls (close the accumulation groups)
                nc.tensor.matmul(poEO[:, 0], biasm[0:64], delta[0:64],
                                 start=False, stop=True, tile_position=(0, 0))
                nc.tensor.matmul(poEO[:, 1], biasm[64:128], delta[64:128],
                                 start=False, stop=True, tile_position=(64, 0))
                # fused relu + evacuation, then contiguous n-major DMA out
                obEO = opool.tile([128, 2, 512], dt, tag="obEO")
                ha, hb = (0, 1) if ch % 2 == 0 else (1, 0)
                nc.scalar.activation(obEO[:, ha], poEO[:, ha],
                                     mybir.ActivationFunctionType.Relu)
                nc.vector.tensor_scalar_max(obEO[:, hb], poEO[:, hb], 0.0)
                eng = nc.gpsimd if ch % 2 == 0 else nc.sync
                eng.dma_start(ov[ch - 2],
                              obEO[:].rearrange("p g n -> p (g n)"))
                del vhist[ch - 2]

    nc.compile()
    return nc


def _prep_weights(core0, core1, core2, core3, bias):
    import ml_dtypes

    # w01[(m1,m2),(n1,n2),s2]; einsum dims: m=m1 n=n1 s=s1 / o=m2 p=n2 r=s2
    w01 = np.einsum("mns,sopr->monpr", core0[0], core1).reshape(64, 64, 8)
    # w23[s2,(m3,m4),(n3,n4)]; s m=m3 n=n3 t=s3 / p=m4 q=n4
    w23 = np.einsum("smnt,tpq->smpnq", core2, core3[:, :, :, 0]).reshape(8, 64, 64)

    w01r = w01.transpose(0, 2, 1).reshape(64, 512)        # [m12, (s2 n12)]
    w01r = np.concatenate([w01r, w01r], 0).astype(ml_dtypes.bfloat16)
    w23r = w23.transpose(1, 0, 2)                          # [m34, s2, n34]
    w23r = np.concatenate([w23r, w23r], 0).astype(ml_dtypes.bfloat16)

    bm = bias.reshape(64, 64)                              # [n12, n34]
    bm = np.broadcast_to(bm[:, None, :], (64, 2, 64))      # dup across bl
    biasm = np.ascontiguousarray(
        np.concatenate([bm, bm], 0)).astype(ml_dtypes.bfloat16)
    dl = np.zeros((64, R, 64), np.float32)                 # delta[n12, t, j]
    idx = np.arange(64)
    dl[idx, :, idx] = 1.0
    delta = np.ascontiguousarray(
        np.concatenate([dl, dl], 0)).astype(ml_dtypes.bfloat16)
    return w01r, w23r, biasm, delta


def _prep_x(x_core):
    """[2048, 4096] fp32 -> [128, 64, 8, 2, 64] bf16 with
    p = half*64 + m12, row = ch*32 + r*4 + half*2 + bl, col = m12*64 + m34."""
    import ml_dtypes
    xr = x_core.reshape(NCH, NRND, 2, 2, 64, 64)   # ch r h bl m12 m34
    xr = xr.transpose(2, 4, 0, 1, 3, 5)            # h m12 ch r bl m34
    xr = np.ascontiguousarray(xr, np.float32).astype(ml_dtypes.bfloat16)
    return xr.reshape(128, NCH, NRND, 2, 64)


def _unpack_out(o_core):
    """[64, 128, 1024] fp32 device layout -> [2048, 4096] fp32.
    o[ch, q*64+n34, g*512 + t*64 + n12] = out[ch*32 + t*4 + sub, n12*64+n34]
    with sub = {(g,q)}: (0,0)->0 (0,1)->1 (1,1)->2 (1,0)->3."""
    od = o_core.reshape(NCH, 2, 64, 2, NRND, 64)   # ch q n34 g t n12
    res = np.empty((NCH, NRND, 4, 64, 64), np.float32)  # ch t sub n12 n34
    res[:, :, 0] = od[:, 0, :, 0].transpose(0, 2, 3, 1)
    res[:, :, 1] = od[:, 1, :, 0].transpose(0, 2, 3, 1)
    res[:, :, 2] = od[:, 1, :, 1].transpose(0, 2, 3, 1)
    res[:, :, 3] = od[:, 0, :, 1].transpose(0, 2, 3, 1)
    return res.reshape(B, F)


def kernel(x, core0, core1, core2, core3, bias):
    global _compiled
    from concourse.bass_utils import run_bass_kernel_spmd

    if _compiled is None:
        _compiled = _build()
    nc = _compiled

    w01r, w23r, biasm, delta = _prep_weights(
        np.asarray(core0, np.float32), np.asarray(core1, np.float32),
        np.asarray(core2, np.float32), np.asarray(core3, np.float32),
        np.asarray(bias, np.float32))

    x = np.asarray(x, np.float32)
    in_maps = [{
        "x": _prep_x(x[c * B:(c + 1) * B]),
        "w01": w01r, "w23": w23r, "biasm": biasm, "delta": delta,
    } for c in range(NCORES)]
    res = run_bass_kernel_spmd(nc, in_maps, list(range(NCORES)))
    globals()["_last_results"] = res
    out = np.concatenate(
        [_unpack_out(np.asarray(res.results[c]["out"])) for c in range(NCORES)],
        axis=0)
    return out.astype(np.float32)


# revision 12
# speedup vs baseline: 1.0154x; 1.0154x over previous
"""TT-matrix dense layer (KerasDense via tensor-train) on 8 TRN2 NeuronCores.

out[b, :] = relu(x[b, :] @ W + bias),  W = TT(core0..core3), 4096x4096.

Cores merged pairwise (s2 = middle TT bond, rank 8):
  W01[(m1 m2), s2, (n1 n2)] = sum_s1 core0[0,m1,n1,s1] * core1[s1,m2,n2,s2]
  W23[s2, (m3 m4), (n3 n4)] = sum_s3 core2[s2,m3,n3,s3] * core3[s3,m4,n4,0]
  out[b, (n12 n34)] = relu( sum_{s2,m12,m34}
        x[b, (m12 m34)] * W01[m12, s2, n12] * W23[s2, m34, n34] + bias )

Per-core dataflow (batch sharded 8 ways, 2048 rows/core, 64 chunks x 32 rows):
  x is pre-laid-out on the HOST as bf16 [128, ch, r, bl, m34] with
  partition p = half*64 + m12 (half: E rows 4r+{0,1}, O rows 4r+{2,3}),
  so the DMA in is one fully-contiguous stream per 8-chunk super-tile.

  step1 (contract m12), per round r: two row-packed K=64 matmuls
  (x-E stationary at rows 0-63, x-O at rows 64-127), both N=512
  (rhs = W01 [m12, (s2 n12)], duplicated across partition halves),
  writing the two halves of one [128, 1024] 2-bank PSUM tile (pool
  depth 3 = 6 banks so the PE runs ahead of evacuation).
  Evacuate with ONE [128,1024] fp32->bf16 copy, DVE/ACT balanced 5/11.

  step2 (contract m34, PSUM-accumulate s2): 4-quadrant tile_position
  packing -- (0,0)/(64,64) serve the E half, (0,64)/(64,0) the O half,
  all four concurrent (full PE array).  Bias is folded in as a final
  accumulating matmul: lhsT = bias[n12, n34], rhs = delta(n12) pattern.
  Relu is fused into the step2 PSUM->SBUF evacuation; output goes out
  n-major (partitions = (bl, n34)) as contiguous DMA; the HOST inverts
  the layout during the gather.
"""

import numpy as np

BATCH = 16384
NCORES = 8
B = BATCH // NCORES     # 2048 rows per core
F = 4096
R = 8                   # TT bond s2
NCH = 64                # chunks per core, 32 rows each
NRND = 8                # rounds per chunk, 4 rows each
SUP = 8                 # chunks per x super-tile (256 rows)

_compiled = None


def _build():
    from contextlib import ExitStack
    from concourse import bacc, tile, mybir

    dt = mybir.dt.float32
    db = mybir.dt.bfloat16
    nc = bacc.Bacc("TRN2", target_bir_lowering=False, debug=False)

    x_d = nc.dram_tensor("x", [128, NCH, NRND, 2, 64], db, kind="ExternalInput")
    w01_d = nc.dram_tensor("w01", [128, 512], db, kind="ExternalInput")
    w23_d = nc.dram_tensor("w23", [128, R, 64], db, kind="ExternalInput")
    biasm_d = nc.dram_tensor("biasm", [128, 2, 64], db, kind="ExternalInput")
    delta_d = nc.dram_tensor("delta", [128, R, 64], db, kind="ExternalInput")
    out_d = nc.dram_tensor("out", [NCH, 128, 1024], dt, kind="ExternalOutput")

    with tile.TileContext(nc) as tc, ExitStack() as ctx:
        const = ctx.enter_context(tc.tile_pool(name="const", bufs=1))
        xpool = ctx.enter_context(tc.tile_pool(name="x", bufs=2))
        vpool = ctx.enter_context(tc.tile_pool(name="v", bufs=4))
        opool = ctx.enter_context(tc.tile_pool(name="o", bufs=2))
        ps1 = ctx.enter_context(tc.tile_pool(name="ps1", bufs=3, space="PSUM"))
        ps2 = ctx.enter_context(tc.tile_pool(name="ps2", bufs=1, space="PSUM"))

        w01 = const.tile([128, 512], db)
        w23 = const.tile([128, R, 64], db)
        biasm = const.tile([128, 2, 64], db)
        delta = const.tile([128, R, 64], db)
        nc.sync.dma_start(w01[:], w01_d.ap())
        nc.sync.dma_start(w23[:], w23_d.ap())
        nc.sync.dma_start(biasm[:], biasm_d.ap())
        nc.sync.dma_start(delta[:], delta_d.ap())

        xv = x_d.ap()
        ov = out_d.ap()

        # prefetch super-tile 0
        xts = {}
        xt0 = xpool.tile([128, SUP, NRND, 2, 64], db, tag="xt")
        nc.sync.dma_start(xt0[:], xv[:, 0:SUP])
        xts[0] = xt0

        vhist = {}
        ecnt = 0  # global evac instruction counter for DVE/ACT load balance

        def evac_copy(dst, src):
            nonlocal ecnt
            # DVE ~1222ns vs ACT ~1114ns per [128,1024] copy -> DVE 478/1000
            if (ecnt * 478) % 1000 < 478:
                nc.vector.tensor_copy(dst, src)
            else:
                nc.scalar.copy(dst, src)
            ecnt += 1

        for ch in range(NCH + 2):
            if ch < NCH:
                st, chl = divmod(ch, SUP)
                if chl == 0 and st + 1 < NCH // SUP:
                    xtn = xpool.tile([128, SUP, NRND, 2, 64], db, tag="xt")
                    nc.sync.dma_start(xtn[:], xv[:, (st + 1) * SUP:(st + 2) * SUP])
                    xts[st + 1] = xtn
                xt = xts[st]
                vEO = vpool.tile([128, NRND, 1024], db, tag="vEO")
                vhist[ch] = vEO
            if ch >= 2:
                pv = vhist[ch - 2]
                v5 = pv[:].rearrange("p t (g s n) -> p g t s n", g=2, s=R)
                poEO = ps2.tile([128, 2, 512], dt, tag="poEO")

            for r in range(NRND):
                if ch < NCH:
                    # ---- step 1: two row-packed matmuls into one 2-bank tile
                    pEO = ps1.tile([128, 1024], dt, tag="pEO")
                    nc.tensor.matmul(pEO[:, 0:512], xt[0:64, chl, r],
                                     w01[0:64], start=True, stop=True,
                                     tile_position=(0, 0))
                    nc.tensor.matmul(pEO[:, 512:1024], xt[64:128, chl, r],
                                     w01[64:128], start=True, stop=True,
                                     tile_position=(64, 0))
                    evac_copy(vEO[:, r], pEO[:])
                if ch >= 2:
                    # ---- step 2: 4-quadrant packed, accumulate over s2
                    s2 = r
                    st1 = (s2 == 0)
                    nc.tensor.matmul(poEO[0:64, 0], w23[0:64, s2],
                                     v5[0:64, 0, :, s2], start=st1, stop=False,
                                     tile_position=(0, 0))
                    nc.tensor.matmul(poEO[64:128, 0], w23[64:128, s2],
                                     v5[64:128, 0, :, s2], start=st1, stop=False,
                                     tile_position=(64, 64))
                    nc.tensor.matmul(poEO[64:128, 1], w23[0:64, s2],
                                     v5[0:64, 1, :, s2], start=st1, stop=False,
                                     tile_position=(0, 64))
                    nc.tensor.matmul(poEO[0:64, 1], w23[64:128, s2],
                                     v5[64:128, 1, :, s2], start=st1, stop=False,
                                     tile_position=(64, 0))

            if ch >= 2:
                # bias via delta-pattern matmuls (close the accumulation groups)
                nc.tensor.matmul(poEO[:, 0], biasm[0:64], delta[0:64],
                                 start=False, stop=True, tile_position=(0, 0))
                nc.tensor.matmul(poEO[:, 1], biasm[64:128], delta[64:128],
                                 start=False, stop=True, tile_position=(64, 0))
                # fused relu + evacuation, then contiguous n-major DMA out
                obEO = opool.tile([128, 2, 512], dt, tag="obEO")
                if ch % 2 == 0:
                    nc.scalar.activation(obEO[:], poEO[:],
                                         mybir.ActivationFunctionType.Relu)
                else:
                    nc.vector.tensor_scalar_max(obEO[:], poEO[:], 0.0)
                eng = nc.gpsimd if ch % 2 == 0 else nc.sync
                eng.dma_start(ov[ch - 2],
                              obEO[:].rearrange("p g n -> p (g n)"))
                del vhist[ch - 2]

    nc.compile()
    return nc


def _prep_weights(core0, core1, core2, core3, bias):
    import ml_dtypes

    # w01[(m1,m2),(n1,n2),s2]; einsum dims: m=m1 n=n1 s=s1 / o=m2 p=n2 r=s2
    w01 = np.einsum("mns,sopr->monpr", core0[0], core1).reshape(64, 64, 8)
    # w23[s2,(m3,m4),(n3,n4)]; s m=m3 n=n3 t=s3 / p=m4 q=n4
    w23 = np.einsum("smnt,tpq->smpnq", core2, core3[:, :, :, 0]).reshape(8, 64, 64)

    w01r = w01.transpose(0, 2, 1).reshape(64, 512)        # [m12, (s2 n12)]
    w01r = np.concatenate([w01r, w01r], 0).astype(ml_dtypes.bfloat16)
    w23r = w23.transpose(1, 0, 2)                          # [m34, s2, n34]
    w23r = np.concatenate([w23r, w23r], 0).astype(ml_dtypes.bfloat16)

    bm = bias.reshape(64, 64)                              # [n12, n34]
    bm = np.broadcast_to(bm[:, None, :], (64, 2, 64))      # dup across bl
    biasm = np.ascontiguousarray(
        np.concatenate([bm, bm], 0)).astype(ml_dtypes.bfloat16)
    dl = np.zeros((64, R, 64), np.float32)                 # delta[n12, t, j]
    idx = np.arange(64)
    dl[idx, :, idx] = 1.0
    delta = np.ascontiguousarray(
        np.concatenate([dl, dl], 0)).astype(ml_dtypes.bfloat16)
    return w01r, w23r, biasm, delta


def _prep_x(x_core):
    """[2048, 4096] fp32 -> [128, 64, 8, 2, 64] bf16 with
    p = half*64 + m12, row = ch*32 + r*4 + half*2 + bl, col = m12*64 + m34."""
    import ml_dtypes
    xr = x_core.reshape(NCH, NRND, 2, 2, 64, 64)   # ch r h bl m12 m34
    xr = xr.transpose(2, 4, 0, 1, 3, 5)            # h m12 ch r bl m34
    xr = np.ascontiguousarray(xr, np.float32).astype(ml_dtypes.bfloat16)
    return xr.reshape(128, NCH, NRND, 2, 64)


def _unpack_out(o_core):
    """[64, 128, 1024] fp32 device layout -> [2048, 4096] fp32.
    o[ch, q*64+n34, g*512 + t*64 + n12] = out[ch*32 + t*4 + sub, n12*64+n34]
    with sub = {(g,q)}: (0,0)->0 (0,1)->1 (1,1)->2 (1,0)->3."""
    od = o_core.reshape(NCH, 2, 64, 2, NRND, 64)   # ch q n34 g t n12
    res = np.empty((NCH, NRND, 4, 64, 64), np.float32)  # ch t sub n12 n34
    res[:, :, 0] = od[:, 0, :, 0].transpose(0, 2, 3, 1)
    res[:, :, 1] = od[:, 1, :, 0].transpose(0, 2, 3, 1)
    res[:, :, 2] = od[:, 1, :, 1].transpose(0, 2, 3, 1)
    res[:, :, 3] = od[:, 0, :, 1].transpose(0, 2, 3, 1)
    return res.reshape(B, F)


def kernel(x, core0, core1, core2, core3, bias):
    global _compiled
    from concourse.bass_utils import run_bass_kernel_spmd

    if _compiled is None:
        _compiled = _build()
    nc = _compiled

    w01r, w23r, biasm, delta = _prep_weights(
        np.asarray(core0, np.float32), np.asarray(core1, np.float32),
        np.asarray(core2, np.float32), np.asarray(core3, np.float32),
        np.asarray(bias, np.float32))

    x = np.asarray(x, np.float32)
    in_maps = [{
        "x": _prep_x(x[c * B:(c + 1) * B]),
        "w01": w01r, "w23": w23r, "biasm": biasm, "delta": delta,
    } for c in range(NCORES)]
    res = run_bass_kernel_spmd(nc, in_maps, list(range(NCORES)))
    globals()["_last_results"] = res
    out = np.concatenate(
        [_unpack_out(np.asarray(res.results[c]["out"])) for c in range(NCORES)],
        axis=0)
    return out.astype(np.float32)


# revision 13
# speedup vs baseline: 1.0207x; 1.0052x over previous
"""TT-matrix dense layer (KerasDense via tensor-train) on 8 TRN2 NeuronCores.

out[b, :] = relu(x[b, :] @ W + bias),  W = TT(core0..core3), 4096x4096.

Cores merged pairwise (s2 = middle TT bond, rank 8):
  W01[(m1 m2), s2, (n1 n2)] = sum_s1 core0[0,m1,n1,s1] * core1[s1,m2,n2,s2]
  W23[s2, (m3 m4), (n3 n4)] = sum_s3 core2[s2,m3,n3,s3] * core3[s3,m4,n4,0]
  out[b, (n12 n34)] = relu( sum_{s2,m12,m34}
        x[b, (m12 m34)] * W01[m12, s2, n12] * W23[s2, m34, n34] + bias )

Per-core dataflow (batch sharded 8 ways, 2048 rows/core, 64 chunks x 32 rows):
  x is pre-laid-out on the HOST as bf16 [128, ch, r, bl, m34] with
  partition p = half*64 + m12 (half: E rows 4r+{0,1}, O rows 4r+{2,3}),
  so the DMA in is one fully-contiguous stream per 8-chunk super-tile.

  step1 (contract m12), per round r: two row-packed K=64 matmuls
  (x-E stationary at rows 0-63, x-O at rows 64-127), both N=512
  (rhs = W01 [m12, (s2 n12)], duplicated across partition halves),
  writing the two halves of one [128, 1024] 2-bank PSUM tile (pool
  depth 3 = 6 banks so the PE runs ahead of evacuation).
  Evacuate with ONE [128,1024] fp32->bf16 copy, DVE/ACT balanced 5/11.

  step2 (contract m34, PSUM-accumulate s2): 4-quadrant tile_position
  packing -- (0,0)/(64,64) serve the E half, (0,64)/(64,0) the O half,
  all four concurrent (full PE array).  Bias is folded in as a final
  accumulating matmul: lhsT = bias[n12, n34], rhs = delta(n12) pattern.
  Relu is fused into the step2 PSUM->SBUF evacuation; output goes out
  n-major (partitions = (bl, n34)) as contiguous DMA; the HOST inverts
  the layout during the gather.
"""

import numpy as np

BATCH = 16384
NCORES = 8
B = BATCH // NCORES     # 2048 rows per core
F = 4096
R = 8                   # TT bond s2
NCH = 64                # chunks per core, 32 rows each
NRND = 8                # rounds per chunk, 4 rows each
SUP = 8                 # chunks per x super-tile (256 rows)

_compiled = None


def _build():
    from contextlib import ExitStack
    from concourse import bacc, tile, mybir

    dt = mybir.dt.float32
    db = mybir.dt.bfloat16
    nc = bacc.Bacc("TRN2", target_bir_lowering=False, debug=False)

    x_d = nc.dram_tensor("x", [128, NCH, NRND, 2, 64], db, kind="ExternalInput")
    w01_d = nc.dram_tensor("w01", [128, 512], db, kind="ExternalInput")
    w23_d = nc.dram_tensor("w23", [128, R, 64], db, kind="ExternalInput")
    biasm_d = nc.dram_tensor("biasm", [128, 2, 64], db, kind="ExternalInput")
    delta_d = nc.dram_tensor("delta", [128, R, 64], db, kind="ExternalInput")
    out_d = nc.dram_tensor("out", [NCH, 128, 1024], dt, kind="ExternalOutput")

    with tile.TileContext(nc) as tc, ExitStack() as ctx:
        const = ctx.enter_context(tc.tile_pool(name="const", bufs=1))
        xpool = ctx.enter_context(tc.tile_pool(name="x", bufs=2))
        vpool = ctx.enter_context(tc.tile_pool(name="v", bufs=4))
        opool = ctx.enter_context(tc.tile_pool(name="o", bufs=2))
        ps1 = ctx.enter_context(tc.tile_pool(name="ps1", bufs=3, space="PSUM"))
        ps2 = ctx.enter_context(tc.tile_pool(name="ps2", bufs=1, space="PSUM"))

        xv = x_d.ap()
        ov = out_d.ap()

        # super-tile 0 first, sliced per chunk, so chunk-0 compute starts
        # as early as possible; weights go on the gpsimd (SWDGE) queue in
        # parallel with it
        xts = {}
        xt0 = xpool.tile([128, SUP, NRND, 2, 64], db, tag="xt")
        for i in range(SUP):
            nc.sync.dma_start(xt0[:, i], xv[:, i])
        xts[0] = xt0

        w01 = const.tile([128, 512], db)
        w23 = const.tile([128, R, 64], db)
        biasm = const.tile([128, 2, 64], db)
        delta = const.tile([128, R, 64], db)
        nc.gpsimd.dma_start(w01[:], w01_d.ap())
        nc.gpsimd.dma_start(w23[:], w23_d.ap())
        nc.gpsimd.dma_start(biasm[:], biasm_d.ap())
        nc.gpsimd.dma_start(delta[:], delta_d.ap())

        vhist = {}
        ecnt = 0  # global evac instruction counter for DVE/ACT load balance

        def evac_copy(dst, src):
            nonlocal ecnt
            # DVE ~1222ns vs ACT ~1114ns per [128,1024] copy -> DVE 478/1000
            if (ecnt * 478) % 1000 < 478:
                nc.vector.tensor_copy(dst, src)
            else:
                nc.scalar.copy(dst, src)
            ecnt += 1

        for ch in range(NCH + 2):
            if ch < NCH:
                st, chl = divmod(ch, SUP)
                if chl == 0 and st + 1 < NCH // SUP:
                    xtn = xpool.tile([128, SUP, NRND, 2, 64], db, tag="xt")
                    nc.sync.dma_start(xtn[:], xv[:, (st + 1) * SUP:(st + 2) * SUP])
                    xts[st + 1] = xtn
                xt = xts[st]
                vEO = vpool.tile([128, NRND, 1024], db, tag="vEO")
                vhist[ch] = vEO
            if ch >= 2:
                pv = vhist[ch - 2]
                v5 = pv[:].rearrange("p t (g s n) -> p g t s n", g=2, s=R)
                poEO = ps2.tile([128, 2, 512], dt, tag="poEO")

            for r in range(NRND):
                if ch < NCH:
                    # ---- step 1: two row-packed matmuls into one 2-bank tile
                    pEO = ps1.tile([128, 1024], dt, tag="pEO")
                    nc.tensor.matmul(pEO[:, 0:512], xt[0:64, chl, r],
                                     w01[0:64], start=True, stop=True,
                                     tile_position=(0, 0))
                    nc.tensor.matmul(pEO[:, 512:1024], xt[64:128, chl, r],
                                     w01[64:128], start=True, stop=True,
                                     tile_position=(64, 0))
                    evac_copy(vEO[:, r], pEO[:])
                if ch >= 2:
                    # ---- step 2: 4-quadrant packed, accumulate over s2
                    s2 = r
                    st1 = (s2 == 0)
                    nc.tensor.matmul(poEO[0:64, 0], w23[0:64, s2],
                                     v5[0:64, 0, :, s2], start=st1, stop=False,
                                     tile_position=(0, 0))
                    nc.tensor.matmul(poEO[64:128, 0], w23[64:128, s2],
                                     v5[64:128, 0, :, s2], start=st1, stop=False,
                                     tile_position=(64, 64))
                    nc.tensor.matmul(poEO[64:128, 1], w23[0:64, s2],
                                     v5[0:64, 1, :, s2], start=st1, stop=False,
                                     tile_position=(0, 64))
                    nc.tensor.matmul(poEO[0:64, 1], w23[64:128, s2],
                                     v5[64:128, 1, :, s2], start=st1, stop=False,
                                     tile_position=(64, 0))

            if ch >= 2:
                # bias via delta-pattern matmuls (close the accumulation groups)
                nc.tensor.matmul(poEO[:, 0], biasm[0:64], delta[0:64],
                                 start=False, stop=True, tile_position=(0, 0))
                nc.tensor.matmul(poEO[:, 1], biasm[64:128], delta[64:128],
                                 start=False, stop=True, tile_position=(64, 0))
                # fused relu + evacuation, then contiguous n-major DMA out
                obEO = opool.tile([128, 2, 512], dt, tag="obEO")
                if ch % 2 == 0:
                    nc.scalar.activation(obEO[:], poEO[:],
                                         mybir.ActivationFunctionType.Relu)
                else:
                    nc.vector.tensor_scalar_max(obEO[:], poEO[:], 0.0)
                eng = nc.gpsimd if ch % 2 == 0 else nc.sync
                eng.dma_start(ov[ch - 2],
                              obEO[:].rearrange("p g n -> p (g n)"))
                del vhist[ch - 2]

    nc.compile()
    return nc


def _prep_weights(core0, core1, core2, core3, bias):
    import ml_dtypes

    # w01[(m1,m2),(n1,n2),s2]; einsum dims: m=m1 n=n1 s=s1 / o=m2 p=n2 r=s2
    w01 = np.einsum("mns,sopr->monpr", core0[0], core1).reshape(64, 64, 8)
    # w23[s2,(m3,m4),(n3,n4)]; s m=m3 n=n3 t=s3 / p=m4 q=n4
    w23 = np.einsum("smnt,tpq->smpnq", core2, core3[:, :, :, 0]).reshape(8, 64, 64)

    w01r = w01.transpose(0, 2, 1).reshape(64, 512)        # [m12, (s2 n12)]
    w01r = np.concatenate([w01r, w01r], 0).astype(ml_dtypes.bfloat16)
    w23r = w23.transpose(1, 0, 2)                          # [m34, s2, n34]
    w23r = np.concatenate([w23r, w23r], 0).astype(ml_dtypes.bfloat16)

    bm = bias.reshape(64, 64)                              # [n12, n34]
    bm = np.broadcast_to(bm[:, None, :], (64, 2, 64))      # dup across bl
    biasm = np.ascontiguousarray(
        np.concatenate([bm, bm], 0)).astype(ml_dtypes.bfloat16)
    dl = np.zeros((64, R, 64), np.float32)                 # delta[n12, t, j]
    idx = np.arange(64)
    dl[idx, :, idx] = 1.0
    delta = np.ascontiguousarray(
        np.concatenate([dl, dl], 0)).astype(ml_dtypes.bfloat16)
    return w01r, w23r, biasm, delta


def _prep_x(x_core):
    """[2048, 4096] fp32 -> [128, 64, 8, 2, 64] bf16 with
    p = half*64 + m12, row = ch*32 + r*4 + half*2 + bl, col = m12*64 + m34."""
    import ml_dtypes
    xr = x_core.reshape(NCH, NRND, 2, 2, 64, 64)   # ch r h bl m12 m34
    xr = xr.transpose(2, 4, 0, 1, 3, 5)            # h m12 ch r bl m34
    xr = np.ascontiguousarray(xr, np.float32).astype(ml_dtypes.bfloat16)
    return xr.reshape(128, NCH, NRND, 2, 64)


def _unpack_out(o_core):
    """[64, 128, 1024] fp32 device layout -> [2048, 4096] fp32.
    o[ch, q*64+n34, g*512 + t*64 + n12] = out[ch*32 + t*4 + sub, n12*64+n34]
    with sub = {(g,q)}: (0,0)->0 (0,1)->1 (1,1)->2 (1,0)->3."""
    od = o_core.reshape(NCH, 2, 64, 2, NRND, 64)   # ch q n34 g t n12
    res = np.empty((NCH, NRND, 4, 64, 64), np.float32)  # ch t sub n12 n34
    res[:, :, 0] = od[:, 0, :, 0].transpose(0, 2, 3, 1)
    res[:, :, 1] = od[:, 1, :, 0].transpose(0, 2, 3, 1)
    res[:, :, 2] = od[:, 1, :, 1].transpose(0, 2, 3, 1)
    res[:, :, 3] = od[:, 0, :, 1].transpose(0, 2, 3, 1)
    return res.reshape(B, F)


def kernel(x, core0, core1, core2, core3, bias):
    global _compiled
    from concourse.bass_utils import run_bass_kernel_spmd

    if _compiled is None:
        _compiled = _build()
    nc = _compiled

    w01r, w23r, biasm, delta = _prep_weights(
        np.asarray(core0, np.float32), np.asarray(core1, np.float32),
        np.asarray(core2, np.float32), np.asarray(core3, np.float32),
        np.asarray(bias, np.float32))

    x = np.asarray(x, np.float32)
    in_maps = [{
        "x": _prep_x(x[c * B:(c + 1) * B]),
        "w01": w01r, "w23": w23r, "biasm": biasm, "delta": delta,
    } for c in range(NCORES)]
    res = run_bass_kernel_spmd(nc, in_maps, list(range(NCORES)))
    globals()["_last_results"] = res
    out = np.concatenate(
        [_unpack_out(np.asarray(res.results[c]["out"])) for c in range(NCORES)],
        axis=0)
    return out.astype(np.float32)
